# revision 1
# baseline (speedup 1.0000x reference)
"""HyperGNN message-passing kernel (nn_Conv_13778255086166) for 8 TRN2 NeuronCores.

Reference computation:
    Xp    = X @ W                                   [N, 64]
    Xe_s  = segment_sum(Xp[vertex], edges, E);  cnt = segment_sum(1, edges, E)
    Ye    = (homo / max(cnt,1)) * Xe_s              [E, 64]   (mean aggregation * homo)
    att_s = segment_sum(homo[edges], vertex, N)
    Xv    = segment_sum(Ye[edges], vertex, N) / att_s
    out   = row_l2_normalize(Xp + Xv)

Distribution (graph parallelism per the sharding hint): the incidence list is
sharded by vertex range — core k owns nodes [k*12500, (k+1)*12500) and all
incidences whose vertex falls in that range.  Per core:

  phase 0: Xp slice = X_local @ W -> DRAM table XpD [12544, 64]
  phase 1: per 128-edge tile, dma_gather the Xp rows of the tile's
           (host-sorted, padded) incidence slots, and accumulate them into
           PSUM with TensorE one-hot matmuls (selection matrix built on DVE
           from slot offsets); a parallel ones-matmul accumulates cnt.
           -> local partial Eacc [25088, 65] ([sums | cnt])
  AllReduce(Eacc) over the 8 cores -> Ered
  Ze build: Ze[:, 0:64] = Ered[:, 0:64] * homo / max(cnt, 1); Ze[:, 64] = homo
           -> ZeF [25088, 128] (512B rows; cols 65:127 never read)
  phase 2: per 128-node tile, dma_gather ZeF rows of the vertex-sorted slots,
           one-hot matmul -> PSUM [128, 65] = [sum Ye | att_sum]; finalize
           Xv = S * recip(max(att, eps)); out = (Xp + Xv) * recip(rownorm)
           -> out slice [12544, 64]; host concatenates the 8 node slices.

All arithmetic (matmul, all segment sums, normalizations) runs on device.
The host only reorganizes the incidence lists (shard by vertex range, order
by segment, pad to fixed per-tile capacity) and formats index tensors —
schedule/layout preparation, not computation.
"""

from dataclasses import dataclass

import numpy as np

import concourse.bacc as bacc
import concourse.mybir as mybir
import concourse.tile as tile
from concourse import bass_utils

F32 = mybir.dt.float32
I16 = mybir.dt.int16


@dataclass(frozen=True)
class Cfg:
    n_cores: int = 8
    N: int = 100000
    E: int = 25000
    cap1: int = 1536   # incidence slots per 128-edge tile per core (mult of 128)
    cap2: int = 3072   # incidence slots per 128-node tile per core (mult of 128)

    @property
    def npc(self):
        assert self.N % self.n_cores == 0
        return self.N // self.n_cores

    @property
    def npcp(self):  # padded, with at least one spare zero row
        return (self.npc + 1 + 127) // 128 * 128

    @property
    def ntiles(self):
        return self.npcp // 128

    @property
    def ep(self):
        return (self.E + 1 + 127) // 128 * 128

    @property
    def etiles(self):
        return self.ep // 128


def wrap_idx(idx: np.ndarray) -> np.ndarray:
    """int16 index layout for dma_gather: element j at [j%16, j//16],
    replicated across the 8 16-partition groups (one per Q7 cpu)."""
    s = idx.shape[0]
    assert s % 16 == 0
    w = np.ascontiguousarray(idx.astype(np.int16).reshape(-1, 16).T)
    return np.tile(w, (8, 1))


def prep_core_inputs(cfg: Cfg, k: int, X, W, homo, vertex, edges):
    """Host-side shard/sort/pad for core k (index/layout reorganization only)."""
    npc, npcp = cfg.npc, cfg.npcp
    sel = (vertex >= k * npc) & (vertex < (k + 1) * npc)
    v_l = (np.asarray(vertex)[sel] - k * npc).astype(np.int64)
    e_l = np.asarray(edges)[sel].astype(np.int64)

    def build(seg, other, tiles_n, cap, pad_gather):
        o = np.argsort(seg, kind="stable")
        s, g = seg[o], other[o]
        t_of = s >> 7
        counts = np.bincount(t_of, minlength=tiles_n)
        assert (counts <= cap).all(), (counts.max(), cap)
        starts = np.cumsum(counts) - counts
        rank = np.arange(len(s)) - starts[t_of]
        dest = t_of * cap + rank
        S = tiles_n * cap
        gi = np.full(S, pad_gather, np.int64)
        off = np.zeros(S, np.float32)
        val = np.zeros(S, np.float32)
        gi[dest] = g
        off[dest] = (s & 127).astype(np.float32)
        val[dest] = 1.0
        return gi, off, val

    # P1: segment by edge, gather by local vertex; pads gather zero row npc.
    g1, off1, val1 = build(e_l, v_l, cfg.etiles, cfg.cap1, pad_gather=npc)
    # P2: segment by local vertex, gather by edge; pads gather zero row E.
    g2, off2, _ = build(v_l, e_l, cfg.ntiles, cfg.cap2, pad_gather=cfg.E)

    def tilemaj_idx(gi, tiles_n, cap):
        w = np.stack([wrap_idx(gi[t * cap:(t + 1) * cap]) for t in range(tiles_n)])
        return np.ascontiguousarray(w)

    def tilemaj_f32(a, tiles_n, cap):
        return np.ascontiguousarray(
            a.reshape(tiles_n, cap // 128, 128).transpose(0, 2, 1))

    Xt = np.zeros((64, npcp), np.float32)
    Xt[:, :npc] = np.asarray(X)[k * npc:(k + 1) * npc].T

    homo_pad = np.zeros(cfg.ep, np.float32)
    homo_pad[:cfg.E] = np.asarray(homo)
    homo_t = np.ascontiguousarray(homo_pad.reshape(cfg.etiles, 128).T)

    iota = np.broadcast_to(np.arange(128, dtype=np.float32), (128, 128)).copy()

    return {
        "Xt": Xt,
        "W": np.asarray(W, dtype=np.float32),
        "homo_t": homo_t,
        "iota": iota,
        "g1": tilemaj_idx(g1, cfg.etiles, cfg.cap1),
        "off1": tilemaj_f32(off1, cfg.etiles, cfg.cap1),
        "val1": tilemaj_f32(val1, cfg.etiles, cfg.cap1),
        "g2": tilemaj_idx(g2, cfg.ntiles, cfg.cap2),
        "off2": tilemaj_f32(off2, cfg.ntiles, cfg.cap2),
    }


def build_nc(cfg: Cfg):
    c1 = cfg.cap1 // 128
    c2 = cfg.cap2 // 128
    nc = bacc.Bacc("TRN2", target_bir_lowering=False, debug=False,
                   num_devices=cfg.n_cores)

    xt_d = nc.dram_tensor("Xt", [64, cfg.npcp], F32, kind="ExternalInput")
    w_d = nc.dram_tensor("W", [64, 64], F32, kind="ExternalInput")
    homo_d = nc.dram_tensor("homo_t", [128, cfg.etiles], F32, kind="ExternalInput")
    iota_d = nc.dram_tensor("iota", [128, 128], F32, kind="ExternalInput")
    g1_d = nc.dram_tensor("g1", [cfg.etiles, 128, cfg.cap1 // 16], I16, kind="ExternalInput")
    off1_d = nc.dram_tensor("off1", [cfg.etiles, 128, c1], F32, kind="ExternalInput")
    val1_d = nc.dram_tensor("val1", [cfg.etiles, 128, c1], F32, kind="ExternalInput")
    g2_d = nc.dram_tensor("g2", [cfg.ntiles, 128, cfg.cap2 // 16], I16, kind="ExternalInput")
    off2_d = nc.dram_tensor("off2", [cfg.ntiles, 128, c2], F32, kind="ExternalInput")
    out_d = nc.dram_tensor("out", [cfg.npcp, 64], F32, kind="ExternalOutput")

    xp_d = nc.dram_tensor("XpD", [cfg.npcp, 64], F32, kind="Internal")
    eacc_d = nc.dram_tensor("EaccD", [cfg.ep, 65], F32, kind="Internal")
    ered_d = nc.dram_tensor("EredD", [cfg.ep, 65], F32, kind="Internal", addr_space="Shared")
    zef_d = nc.dram_tensor("ZeFD", [cfg.ep, 128], F32, kind="Internal")

    with tile.TileContext(nc) as tc:
        with (
            tc.tile_pool(name="const", bufs=1) as pc,
            tc.tile_pool(name="idx", bufs=4) as pidx,
            tc.tile_pool(name="gather", bufs=3) as pg,
            tc.tile_pool(name="onehot", bufs=4) as pm,
            tc.tile_pool(name="sbout", bufs=3) as po,
            tc.tile_pool(name="fin", bufs=4) as pf,
            tc.tile_pool(name="psum", bufs=2, space="PSUM") as pp,
        ):
            xt_sb = pc.tile([64, cfg.npcp], F32)
            nc.sync.dma_start(out=xt_sb[:], in_=xt_d[:])
            w_sb = pc.tile([64, 64], F32)
            nc.sync.dma_start(out=w_sb[:], in_=w_d[:])
            iota_sb = pc.tile([128, 128], F32)
            nc.sync.dma_start(out=iota_sb[:], in_=iota_d[:])
            homo_sb = pc.tile([128, cfg.etiles], F32)
            nc.sync.dma_start(out=homo_sb[:], in_=homo_d[:])

            # phase 0: Xp = X_local @ W
            for t in range(cfg.ntiles):
                ps = pp.tile([128, 64], F32, tag="ps0")
                nc.tensor.matmul(ps[:], lhsT=xt_sb[:, t * 128:(t + 1) * 128],
                                 rhs=w_sb[:], start=True, stop=True)
                xp_sb = po.tile([128, 64], F32, tag="xp0")
                nc.vector.tensor_copy(out=xp_sb[:], in_=ps[:])
                nc.sync.dma_start(out=xp_d[t * 128:(t + 1) * 128, :], in_=xp_sb[:])

            # phase 1: edge-tile accumulation
            for s in range(cfg.etiles):
                gi = pidx.tile([128, cfg.cap1 // 16], I16, tag="gi1")
                nc.sync.dma_start(out=gi[:], in_=g1_d[s])
                of = pidx.tile([128, c1], F32, tag="of1")
                nc.sync.dma_start(out=of[:], in_=off1_d[s])
                vl = pidx.tile([128, c1], F32, tag="vl1")
                nc.sync.dma_start(out=vl[:], in_=val1_d[s])
                g = pg.tile([128, c1, 64], F32, tag="g1")
                nc.gpsimd.dma_gather(g[:], xp_d[:], gi[:], cfg.cap1, cfg.cap1, 64,
                                     single_packet=False)
                ps = pp.tile([128, 64], F32, tag="ps1")
                psc = pp.tile([128, 1], F32, tag="ps1c")
                for j in range(c1):
                    mt = pm.tile([128, 128], F32, tag="mt1")
                    nc.vector.tensor_scalar(out=mt[:], in0=iota_sb[:],
                                            scalar1=of[:, j:j + 1], scalar2=None,
                                            op0=mybir.AluOpType.is_equal)
                    nc.tensor.matmul(ps[:], lhsT=mt[:], rhs=g[:, j, :],
                                     start=(j == 0), stop=(j == c1 - 1))
                    nc.tensor.matmul(psc[:], lhsT=mt[:], rhs=vl[:, j:j + 1],
                                     start=(j == 0), stop=(j == c1 - 1))
                acc = po.tile([128, 65], F32, tag="acc1")
                nc.vector.tensor_copy(out=acc[:, 0:64], in_=ps[:])
                nc.vector.tensor_copy(out=acc[:, 64:65], in_=psc[:])
                nc.sync.dma_start(out=eacc_d[s * 128:(s + 1) * 128, :], in_=acc[:])

            # AllReduce edge partials
            nc.gpsimd.collective_compute(
                "AllReduce", mybir.AluOpType.add,
                replica_groups=[list(range(cfg.n_cores))],
                ins=[eacc_d.ap()], outs=[ered_d.ap()],
            )

            # Ze build: [Ye | homo | zeros]
            for t in range(cfg.etiles):
                er = pf.tile([128, 65], F32, tag="er")
                nc.sync.dma_start(out=er[:], in_=ered_d[t * 128:(t + 1) * 128, :])
                cntm = pf.tile([128, 1], F32, tag="cntm")
                nc.vector.tensor_scalar_max(out=cntm[:], in0=er[:, 64:65], scalar1=1.0)
                rec = pf.tile([128, 1], F32, tag="rec")
                nc.vector.reciprocal(out=rec[:], in_=cntm[:])
                scale = pf.tile([128, 1], F32, tag="scale")
                nc.vector.tensor_tensor(out=scale[:], in0=rec[:],
                                        in1=homo_sb[:, t:t + 1],
                                        op=mybir.AluOpType.mult)
                z = po.tile([128, 128], F32, tag="z")
                nc.vector.memset(z[:, 64:128], 0.0)
                nc.vector.tensor_scalar_mul(out=z[:, 0:64], in0=er[:, 0:64],
                                            scalar1=scale[:])
                nc.vector.tensor_copy(out=z[:, 64:65], in_=homo_sb[:, t:t + 1])
                nc.sync.dma_start(out=zef_d[t * 128:(t + 1) * 128, :], in_=z[:])

            # phase 2: node-tile accumulation + finalize
            for s in range(cfg.ntiles):
                gi = pidx.tile([128, cfg.cap2 // 16], I16, tag="gi2")
                nc.sync.dma_start(out=gi[:], in_=g2_d[s])
                of = pidx.tile([128, c2], F32, tag="of2")
                nc.sync.dma_start(out=of[:], in_=off2_d[s])
                g = pg.tile([128, c2, 128], F32, tag="g2")
                nc.gpsimd.dma_gather(g[:], zef_d[:], gi[:], cfg.cap2, cfg.cap2, 128,
                                     single_packet=False)
                ps = pp.tile([128, 65], F32, tag="ps2")
                for j in range(c2):
                    mt = pm.tile([128, 128], F32, tag="mt2")
                    nc.vector.tensor_scalar(out=mt[:], in0=iota_sb[:],
                                            scalar1=of[:, j:j + 1], scalar2=None,
                                            op0=mybir.AluOpType.is_equal)
                    nc.tensor.matmul(ps[:, 0:65], lhsT=mt[:], rhs=g[:, j, 0:65],
                                     start=(j == 0), stop=(j == c2 - 1))
                attm = pf.tile([128, 1], F32, tag="attm")
                nc.vector.tensor_scalar_max(out=attm[:], in0=ps[:, 64:65], scalar1=1e-30)
                arec = pf.tile([128, 1], F32, tag="arec")
                nc.vector.reciprocal(out=arec[:], in_=attm[:])
                xp_sb = pf.tile([128, 64], F32, tag="xpl")
                nc.sync.dma_start(out=xp_sb[:], in_=xp_d[s * 128:(s + 1) * 128, :])
                o = pf.tile([128, 64], F32, tag="o")
                nc.vector.tensor_scalar_mul(out=o[:], in0=ps[:, 0:64], scalar1=arec[:])
                nc.vector.tensor_tensor(out=o[:], in0=o[:], in1=xp_sb[:],
                                        op=mybir.AluOpType.add)
                sq = pf.tile([128, 64], F32, tag="sq")
                nc.vector.tensor_tensor(out=sq[:], in0=o[:], in1=o[:],
                                        op=mybir.AluOpType.mult)
                rs = pf.tile([128, 1], F32, tag="rs")
                nc.vector.reduce_sum(out=rs[:], in_=sq[:], axis=mybir.AxisListType.X)
                rn = pf.tile([128, 1], F32, tag="rn")
                nc.scalar.sqrt(out=rn[:], in_=rs[:])
                rnm = pf.tile([128, 1], F32, tag="rnm")
                nc.vector.tensor_scalar_max(out=rnm[:], in0=rn[:], scalar1=1e-30)
                rrec = pf.tile([128, 1], F32, tag="rrec")
                nc.vector.reciprocal(out=rrec[:], in_=rnm[:])
                ot = po.tile([128, 64], F32, tag="ot")
                nc.vector.tensor_scalar_mul(out=ot[:], in0=o[:], scalar1=rrec[:])
                nc.sync.dma_start(out=out_d[s * 128:(s + 1) * 128, :], in_=ot[:])

    nc.compile()
    return nc


_NC_CACHE = {}


def kernel(**inputs) -> np.ndarray:
    """Full inputs in, full output out. Shards across 8 NeuronCores internally."""
    cfg = Cfg()
    X = np.asarray(inputs["X"], dtype=np.float32)
    W = np.asarray(inputs["W"], dtype=np.float32)
    homo = np.asarray(inputs["homo"], dtype=np.float32)
    vertex = np.asarray(inputs["vertex"])
    edges = np.asarray(inputs["edges"])
    assert X.shape == (cfg.N, 64) and homo.shape == (cfg.E,)

    key = cfg
    if key not in _NC_CACHE:
        _NC_CACHE[key] = build_nc(cfg)
    nc = _NC_CACHE[key]

    in_maps = [prep_core_inputs(cfg, k, X, W, homo, vertex, edges)
               for k in range(cfg.n_cores)]
    res = bass_utils.run_bass_kernel_spmd(
        nc, in_maps, core_ids=list(range(cfg.n_cores)))
    out = np.concatenate(
        [res.results[k]["out"][:cfg.npc] for k in range(cfg.n_cores)], axis=0)
    return out.astype(np.float32)



# revision 2
# speedup vs baseline: 2.1337x; 2.1337x over previous
"""HyperGNN message-passing kernel (nn_Conv_13778255086166) for 8 TRN2 NeuronCores.

Reference computation:
    Xp    = X @ W                                   [N, 64]
    Xe_s  = segment_sum(Xp[vertex], edges, E);  cnt = segment_sum(1, edges, E)
    Ze    = (homo / max(cnt,1)) * Xe_s              [E, 64]   (mean aggregation * homo)
    att_s = segment_sum(homo[edges], vertex, N)
    Xv    = segment_sum(Ze[edges], vertex, N) / att_s
    out   = row_l2_normalize(Xp + Xv)

v2 design (all arithmetic on device; host only shards/sorts/pads index lists and
formats one-hot selection matrices = pure index/layout prep):

  phase 0 (vertex-sharded): each core computes Xp rows [12544k, 12544(k+1))
      as [XpT | 1] bf16 rows of 128 cols -> AllGather -> XptFull [100352, 128].
  phase 1 (edge-sharded): core k owns edges [3200k, 3200(k+1)) = 25 edge tiles.
      Per tile: dma_gather the incident vertices' Xp rows (4 sub-gathers on
      SWDGE queues 0-3, idx striped by v%4 so int16 indices fit), then 88
      one-hot matmuls (masks streamed bf16 from HBM) accumulate
      [128 edges, 65] = [sums | cnt] in PSUM. Ze = homo/max(cnt,1) * sums
      -> ZetMy [3200, 128] bf16 -> AllGather -> ZetFull [25600, 128].
  phase 2 (vertex-sharded): per 128-node tile, dma_gather ZetFull rows of the
      node-sorted slots (queues round-robin), 22 one-hot matmuls ->
      [128, 65] = [sum homo*Ze | att_sum]; finalize
      out = rownorm(Xp + sums/att_sum) -> out slice; host concats.

The one-hot masks make slot order within a tile irrelevant, absorb padding
(zero rows) and fuse the count (ones column baked into the Xp table).
"""

from dataclasses import dataclass

import numpy as np
import ml_dtypes

import concourse.bacc as bacc
import concourse.mybir as mybir
import concourse.tile as tile
from concourse import bass_utils

F32 = mybir.dt.float32
BF = mybir.dt.bfloat16
I16 = mybir.dt.int16
BF_NP = ml_dtypes.bfloat16


@dataclass(frozen=True)
class Cfg:
    n_cores: int = 8
    N: int = 100000
    E: int = 25000
    NV: int = 100352    # padded vertex rows (= 98 * 1024); NV/8 per core
    EPC: int = 3200     # edges per core -> 25 edge tiles of 128
    EPAD: int = 25600   # 8 * EPC
    cap1: int = 2816    # slots per (edge tile, stripe); 22 chunks of 128
    cap2: int = 2816    # slots per node tile; 22 chunks
    NS: int = 4         # index stripes (v % 4) so int16 gather indices fit

    @property
    def npc(self):
        return self.NV // self.n_cores  # 12544 (node/phase-0 shard)

    @property
    def nt(self):
        return self.npc // 128          # 98 node tiles

    @property
    def et(self):
        return self.EPC // 128          # 25 edge tiles

    @property
    def c1(self):
        return self.cap1 // 128         # 22

    @property
    def ch1(self):
        return self.NS * self.c1        # 88 chunks per edge tile

    @property
    def c2(self):
        return self.cap2 // 128         # 22


def wrap_idx(idx: np.ndarray) -> np.ndarray:
    """int16 index layout for dma_gather: element j at [j%16, j//16],
    replicated across the 8 16-partition groups."""
    s = idx.shape[0]
    assert s % 16 == 0
    w = np.ascontiguousarray(idx.astype(np.int16).reshape(-1, 16).T)
    return np.tile(w, (8, 1))


def prep_core_inputs(cfg: Cfg, k: int, X, W, homo, vertex, edges):
    """Host-side shard/sort/pad for core k (index/layout reorganization only)."""
    N, npc, EPC = cfg.N, cfg.npc, cfg.EPC
    vertex = np.asarray(vertex).astype(np.int64)
    edges = np.asarray(edges).astype(np.int64)

    # ---- phase 0: transposed X slice + ones row, bf16 ----
    xt = np.zeros((65, npc), dtype=BF_NP)
    lo, hi = npc * k, min(npc * (k + 1), N)
    xt[0:64, 0:hi - lo] = np.asarray(X)[lo:hi].T.astype(BF_NP)
    xt[64, :] = BF_NP(1.0)

    w65 = np.zeros((65, 65), dtype=BF_NP)
    w65[0:64, 0:64] = np.asarray(W).astype(BF_NP)
    w65[64, 64] = BF_NP(1.0)

    # ---- phase 1 (edge shard): slots with edge in [EPC*k, EPC*(k+1)) ----
    sel = (edges >= EPC * k) & (edges < EPC * (k + 1))
    e_l = edges[sel] - EPC * k
    v_g = vertex[sel]
    t = e_l >> 7                      # edge tile 0..24
    g = v_g & 3                       # stripe
    r = v_g >> 2                      # within-stripe gather row (< 25088)
    col = e_l & 127
    key = t * cfg.NS + g
    order = np.argsort(key, kind="stable")
    ks, rs_, cols = key[order], r[order], col[order]
    counts = np.bincount(ks, minlength=cfg.et * cfg.NS)
    assert counts.max() <= cfg.cap1, (counts.max(), cfg.cap1)
    starts = np.cumsum(counts) - counts
    rank = np.arange(len(ks)) - starts[ks]
    # gather index array [et, NS, cap1] (pads -> row 0, masked out)
    g1 = np.zeros((cfg.et, cfg.NS, cfg.cap1), np.int64)
    g1[ks // cfg.NS, ks % cfg.NS, rank] = rs_
    # masks [et, 128, ch1*128], slot position within tile = g*cap1 + rank
    pos = (ks % cfg.NS) * cfg.cap1 + rank
    m1 = np.zeros((cfg.et, 128, cfg.ch1 * 128), dtype=BF_NP)
    m1[ks // cfg.NS, pos & 127, (pos >> 7) * 128 + cols] = BF_NP(1.0)

    g1w = np.stack([
        np.concatenate([wrap_idx(g1[tt, gg]) for gg in range(cfg.NS)], axis=1)
        for tt in range(cfg.et)])

    homo_pad = np.zeros(EPC, np.float32)
    e0, e1 = EPC * k, min(EPC * (k + 1), cfg.E)
    homo_pad[0:e1 - e0] = np.asarray(homo)[e0:e1]
    homo_t = np.ascontiguousarray(homo_pad.reshape(cfg.et, 128).T)

    # ---- phase 2 (vertex shard): slots with vertex in [npc*k, npc*(k+1)) ----
    sel2 = (vertex >= npc * k) & (vertex < npc * (k + 1))
    v_l = vertex[sel2] - npc * k
    e_g = edges[sel2]                 # global edge id < 25000 (int16 ok)
    s = v_l >> 7
    col2 = v_l & 127
    order2 = np.argsort(s, kind="stable")
    ss, eg2, c2s = s[order2], e_g[order2], col2[order2]
    counts2 = np.bincount(ss, minlength=cfg.nt)
    assert counts2.max() <= cfg.cap2, (counts2.max(), cfg.cap2)
    starts2 = np.cumsum(counts2) - counts2
    rank2 = np.arange(len(ss)) - starts2[ss]
    g2 = np.zeros((cfg.nt, cfg.cap2), np.int64)
    g2[ss, rank2] = eg2
    m2 = np.zeros((cfg.nt, 128, cfg.c2 * 128), dtype=BF_NP)
    m2[ss, rank2 & 127, (rank2 >> 7) * 128 + c2s] = BF_NP(1.0)
    g2w = np.stack([wrap_idx(g2[tt]) for tt in range(cfg.nt)])

    return {
        "xt": xt,
        "w65": w65,
        "homo_t": homo_t,
        "g1": np.ascontiguousarray(g1w),
        "m1": np.ascontiguousarray(m1),
        "g2": np.ascontiguousarray(g2w),
        "m2": np.ascontiguousarray(m2),
    }


def build_nc(cfg: Cfg):
    c1, c2, ch1 = cfg.c1, cfg.c2, cfg.ch1
    nc = bacc.Bacc("TRN2", target_bir_lowering=False, debug=False,
                   num_devices=cfg.n_cores, num_swdge_queues=4)

    xt_d = nc.dram_tensor("xt", [65, cfg.npc], BF, kind="ExternalInput")
    w_d = nc.dram_tensor("w65", [65, 65], BF, kind="ExternalInput")
    homo_d = nc.dram_tensor("homo_t", [128, cfg.et], F32, kind="ExternalInput")
    g1_d = nc.dram_tensor("g1", [cfg.et, 128, cfg.NS * cfg.cap1 // 16], I16,
                          kind="ExternalInput")
    m1_d = nc.dram_tensor("m1", [cfg.et, 128, ch1 * 128], BF, kind="ExternalInput")
    g2_d = nc.dram_tensor("g2", [cfg.nt, 128, cfg.cap2 // 16], I16,
                          kind="ExternalInput")
    m2_d = nc.dram_tensor("m2", [cfg.nt, 128, c2 * 128], BF, kind="ExternalInput")
    out_d = nc.dram_tensor("out", [cfg.npc, 64], F32, kind="ExternalOutput")

    xptmy_d = nc.dram_tensor("XptMy", [cfg.npc, 128], BF, kind="Internal")
    xptfull_d = nc.dram_tensor("XptFull", [cfg.NV, 128], BF, kind="Internal",
                               addr_space="Shared")
    zetmy_d = nc.dram_tensor("ZetMy", [cfg.EPC, 128], BF, kind="Internal")
    zetfull_d = nc.dram_tensor("ZetFull", [cfg.EPAD, 128], BF, kind="Internal",
                               addr_space="Shared")

    iw = cfg.cap1 // 16  # idx cols per stripe (176)

    with tile.TileContext(nc) as tc:
        with (
            tc.tile_pool(name="const", bufs=1) as pc,
            tc.tile_pool(name="stg", bufs=4) as pstg,
            tc.tile_pool(name="idx", bufs=3) as pidx,
            tc.tile_pool(name="mask", bufs=2) as pm,
            tc.tile_pool(name="gat1", bufs=2) as pg,
            tc.tile_pool(name="gat2", bufs=3) as pg2,
            tc.tile_pool(name="fin", bufs=4) as pf,
            tc.tile_pool(name="zo", bufs=3) as po,
            tc.tile_pool(name="psum", bufs=2, space="PSUM") as pp,
        ):
            xt_sb = pc.tile([65, cfg.npc], BF)
            nc.sync.dma_start(out=xt_sb[:], in_=xt_d[:])
            w_sb = pc.tile([65, 65], BF)
            nc.sync.dma_start(out=w_sb[:], in_=w_d[:])
            homo_sb = pc.tile([128, cfg.et], F32)
            nc.sync.dma_start(out=homo_sb[:], in_=homo_d[:])

            # ---- phase 0: Xp rows for my vertex shard ----
            for s in range(cfg.nt):
                ps = pp.tile([128, 65], F32, tag="ps0")
                nc.tensor.matmul(ps[:], lhsT=xt_sb[:, s * 128:(s + 1) * 128],
                                 rhs=w_sb[:], start=True, stop=True)
                stg = pstg.tile([128, 128], BF, tag="stg")
                nc.vector.tensor_copy(out=stg[:, 0:65], in_=ps[:])
                nc.sync.dma_start(out=xptmy_d[s * 128:(s + 1) * 128, :],
                                  in_=stg[:])

            nc.gpsimd.collective_compute(
                "AllGather", mybir.AluOpType.bypass,
                replica_groups=[list(range(cfg.n_cores))],
                ins=[xptmy_d.ap()], outs=[xptfull_d.ap()],
            )

            # striped views of XptFull: stripe g holds rows v with v%4==g
            stripes = [xptfull_d.ap().rearrange("(r g) c -> g r c", g=cfg.NS)[g]
                       for g in range(cfg.NS)]

            # ---- phase 1: edge-tile accumulation ----
            for t in range(cfg.et):
                gi = pidx.tile([128, cfg.NS * iw], I16, tag="gi1")
                nc.sync.dma_start(out=gi[:], in_=g1_d[t])
                mk = pm.tile([128, ch1 * 128], BF, tag="mk1")
                nc.sync.dma_start(out=mk[:], in_=m1_d[t])
                gs = []
                for g in range(cfg.NS):
                    gt = pg.tile([128, c1, 128], BF, tag=f"g1_{g}")
                    nc.gpsimd.dma_gather(
                        gt[:], stripes[g], gi[:, g * iw:(g + 1) * iw],
                        cfg.cap1, cfg.cap1, 128, elem_step=cfg.NS * 128,
                        single_packet=False, queue_num=g)
                    gs.append(gt)
                ps = pp.tile([128, 65], F32, tag="ps1")
                for c in range(ch1):
                    nc.tensor.matmul(ps[:], lhsT=mk[:, c * 128:(c + 1) * 128],
                                     rhs=gs[c // c1][:, c % c1, 0:65],
                                     start=(c == 0), stop=(c == ch1 - 1))
                er = pf.tile([128, 65], F32, tag="er")
                nc.vector.tensor_copy(out=er[:], in_=ps[:])
                cntm = pf.tile([128, 1], F32, tag="cntm")
                nc.vector.tensor_scalar_max(out=cntm[:], in0=er[:, 64:65],
                                            scalar1=1.0)
                rec = pf.tile([128, 1], F32, tag="rec")
                nc.vector.reciprocal(out=rec[:], in_=cntm[:])
                scale = pf.tile([128, 1], F32, tag="scale")
                nc.vector.tensor_tensor(out=scale[:], in0=rec[:],
                                        in1=homo_sb[:, t:t + 1],
                                        op=mybir.AluOpType.mult)
                z = po.tile([128, 128], BF, tag="z")
                nc.vector.tensor_scalar_mul(out=z[:, 0:64], in0=er[:, 0:64],
                                            scalar1=scale[:])
                nc.vector.tensor_copy(out=z[:, 64:65], in_=homo_sb[:, t:t + 1])
                nc.sync.dma_start(out=zetmy_d[t * 128:(t + 1) * 128, :], in_=z[:])

            nc.gpsimd.collective_compute(
                "AllGather", mybir.AluOpType.bypass,
                replica_groups=[list(range(cfg.n_cores))],
                ins=[zetmy_d.ap()], outs=[zetfull_d.ap()],
            )

            # ---- phase 2: node-tile accumulation + finalize ----
            for s in range(cfg.nt):
                gi = pidx.tile([128, cfg.cap2 // 16], I16, tag="gi2")
                nc.sync.dma_start(out=gi[:], in_=g2_d[s])
                mk = pm.tile([128, c2 * 128], BF, tag="mk2")
                nc.sync.dma_start(out=mk[:], in_=m2_d[s])
                gt = pg2.tile([128, c2, 128], BF, tag="g2")
                nc.gpsimd.dma_gather(gt[:], zetfull_d.ap(), gi[:],
                                     cfg.cap2, cfg.cap2, 128,
                                     single_packet=False, queue_num=s % 4)
                ps = pp.tile([128, 65], F32, tag="ps2")
                for c in range(c2):
                    nc.tensor.matmul(ps[:], lhsT=mk[:, c * 128:(c + 1) * 128],
                                     rhs=gt[:, c, 0:65],
                                     start=(c == 0), stop=(c == c2 - 1))
                attm = pf.tile([128, 1], F32, tag="attm")
                nc.vector.tensor_scalar_max(out=attm[:], in0=ps[:, 64:65],
                                            scalar1=1e-30)
                arec = pf.tile([128, 1], F32, tag="arec")
                nc.vector.reciprocal(out=arec[:], in_=attm[:])
                o = pf.tile([128, 64], F32, tag="o")
                nc.scalar.mul(out=o[:], in_=ps[:, 0:64], mul=arec[:])
                xp_sb = pf.tile([128, 128], BF, tag="xp")
                nc.sync.dma_start(out=xp_sb[:],
                                  in_=xptmy_d[s * 128:(s + 1) * 128, :])
                xpf = pf.tile([128, 64], F32, tag="xpf")
                nc.scalar.copy(out=xpf[:], in_=xp_sb[:, 0:64])
                o2 = pf.tile([128, 64], F32, tag="o2")
                nc.vector.tensor_tensor(out=o2[:], in0=o[:], in1=xpf[:],
                                        op=mybir.AluOpType.add)
                sq = pf.tile([128, 64], F32, tag="sq")
                nc.vector.tensor_tensor(out=sq[:], in0=o2[:], in1=o2[:],
                                        op=mybir.AluOpType.mult)
                rs = pf.tile([128, 1], F32, tag="rs")
                nc.vector.reduce_sum(out=rs[:], in_=sq[:],
                                     axis=mybir.AxisListType.X)
                rn = pf.tile([128, 1], F32, tag="rn")
                nc.scalar.sqrt(out=rn[:], in_=rs[:])
                rnm = pf.tile([128, 1], F32, tag="rnm")
                nc.vector.tensor_scalar_max(out=rnm[:], in0=rn[:], scalar1=1e-30)
                rrec = pf.tile([128, 1], F32, tag="rrec")
                nc.vector.reciprocal(out=rrec[:], in_=rnm[:])
                ot = po.tile([128, 64], F32, tag="ot")
                nc.scalar.mul(out=ot[:], in_=o2[:], mul=rrec[:])
                nc.sync.dma_start(out=out_d[s * 128:(s + 1) * 128, :], in_=ot[:])

    nc.compile()
    return nc


_NC_CACHE = {}


def kernel(**inputs) -> np.ndarray:
    """Full inputs in, full output out. Shards across 8 NeuronCores internally."""
    cfg = Cfg()
    X = np.asarray(inputs["X"], dtype=np.float32)
    W = np.asarray(inputs["W"], dtype=np.float32)
    homo = np.asarray(inputs["homo"], dtype=np.float32)
    vertex = np.asarray(inputs["vertex"])
    edges = np.asarray(inputs["edges"])
    assert X.shape == (cfg.N, 64) and homo.shape == (cfg.E,)

    if cfg not in _NC_CACHE:
        _NC_CACHE[cfg] = build_nc(cfg)
    nc = _NC_CACHE[cfg]

    in_maps = [prep_core_inputs(cfg, k, X, W, homo, vertex, edges)
               for k in range(cfg.n_cores)]
    res = bass_utils.run_bass_kernel_spmd(
        nc, in_maps, core_ids=list(range(cfg.n_cores)))
    out = np.concatenate(
        [res.results[k]["out"] for k in range(cfg.n_cores)], axis=0)
    return out[:cfg.N].astype(np.float32)


# revision 6
# speedup vs baseline: 3.0947x; 1.4504x over previous
"""HyperGNN message-passing kernel (nn_Conv_13778255086166) for 8 TRN2 NeuronCores.

Reference computation:
    Xp    = X @ W                                   [N, 64]
    Xe_s  = segment_sum(Xp[vertex], edges, E);  cnt = segment_sum(1, edges, E)
    Ze    = (homo / max(cnt,1)) * Xe_s              [E, 64]   (mean aggregation * homo)
    att_s = segment_sum(homo[edges], vertex, N)
    Xv    = segment_sum(Ze[edges], vertex, N) / att_s
    out   = row_l2_normalize(Xp + Xv)

v3 design (all arithmetic on device; host only shards/sorts/pads index lists =
pure index/layout prep):

  phase 0 (vertex-sharded): each core computes Xp rows [12544k, 12544(k+1))
      as [XpT | 1] bf16 rows of 128 cols -> AllGather -> XptFull [100352, 128].
  phase 1 (edge-sharded): core k owns edges [3200k, 3200(k+1)) = 25 edge tiles.
      Per tile: dma_gather the incident vertices' Xp rows (4 sub-gathers on
      SWDGE queues 0-3, idx striped by v%4 so int16 indices fit; trailing pads
      are -1 so the Q7 ucode trims them), then 88 one-hot matmuls accumulate
      [128 edges, 65] = [sums | cnt] in PSUM. One-hot masks are built on DVE:
      one is_equal(iota, offset-broadcast) op per 32 chunks. Ze =
      homo/max(cnt,1)*sums -> ZetMy -> AllGather -> ZetFull [25600, 128] bf16.
  phase 2 (vertex-sharded): per 128-node tile, dma_gather ZetFull rows of the
      node slots (4 sub-gathers, queues 0-3), 22 one-hot matmuls ->
      [128, 65] = [sum homo*Ze | att_sum]; finalize
      out = rownorm(Xp + sums/att_sum); host concats the slices.

  Loads issue on the Sync HWDGE queue, stores on the Scalar HWDGE queue so
  dependent stores never head-of-line block prefetch loads.
"""

from dataclasses import dataclass

import numpy as np
import ml_dtypes

import concourse.bacc as bacc
import concourse.mybir as mybir
import concourse.tile as tile
from concourse import bass_utils

F32 = mybir.dt.float32
BF = mybir.dt.bfloat16
I16 = mybir.dt.int16
BF_NP = ml_dtypes.bfloat16


@dataclass(frozen=True)
class Cfg:
    n_cores: int = 8
    N: int = 100000
    E: int = 25000
    NV: int = 100352    # padded vertex rows (= 98 * 1024); NV/8 per core
    EPC: int = 3200     # edges per core -> 25 edge tiles of 128
    EPAD: int = 25600   # 8 * EPC
    cap1: int = 2816    # slots per (edge tile, stripe); 22 chunks of 128
    cap2: int = 2816    # slots per node tile; 22 chunks
    NS: int = 4         # index stripes (v % 4) so int16 gather indices fit
    MW: int = 32        # mask chunks built per DVE is_equal op

    @property
    def npc(self):
        return self.NV // self.n_cores  # 12544 (node/phase-0 shard)

    @property
    def nt(self):
        return self.npc // 128          # 98 node tiles

    @property
    def et(self):
        return self.EPC // 128          # 25 edge tiles

    @property
    def c1(self):
        return self.cap1 // 128         # 22

    @property
    def ch1(self):
        return self.NS * self.c1        # 88 chunks per edge tile

    @property
    def ch1p(self):                     # 88 padded up to MW multiple = 96
        return (self.ch1 + self.MW - 1) // self.MW * self.MW

    @property
    def c2(self):
        return self.cap2 // 128         # 22

    @property
    def c2p(self):                      # 32
        return (self.c2 + self.MW - 1) // self.MW * self.MW


# phase-2 sub-gather split of the 22 chunks across SWDGE queues
P2_SPLIT = (6, 6, 5, 5)
PAD_NEG = False  # -1 trailing pads (ucode trims); False -> pad with 0


def wrap_idx(idx: np.ndarray) -> np.ndarray:
    """int16 index layout for dma_gather: element j at [j%16, j//16],
    replicated across the 8 16-partition groups."""
    s = idx.shape[0]
    assert s % 16 == 0
    w = np.ascontiguousarray(idx.astype(np.int16).reshape(-1, 16).T)
    return np.tile(w, (8, 1))


def prep_core_inputs(cfg: Cfg, k: int, X, W, homo, vertex, edges):
    """Host-side shard/sort/pad for core k (index/layout reorganization only)."""
    N, npc, EPC = cfg.N, cfg.npc, cfg.EPC
    vertex = np.asarray(vertex).astype(np.int64)
    edges = np.asarray(edges).astype(np.int64)

    # ---- phase 0: transposed X slice + ones row, bf16 ----
    xt = np.zeros((65, npc), dtype=BF_NP)
    lo, hi = npc * k, min(npc * (k + 1), N)
    xt[0:64, 0:hi - lo] = np.asarray(X)[lo:hi].T.astype(BF_NP)
    xt[64, :] = BF_NP(1.0)

    w65 = np.zeros((65, 65), dtype=BF_NP)
    w65[0:64, 0:64] = np.asarray(W).astype(BF_NP)
    w65[64, 64] = BF_NP(1.0)

    iota = np.tile(np.arange(128, dtype=np.float32), (128, cfg.MW)).astype(BF_NP)

    # ---- phase 1 (edge shard): slots with edge in [EPC*k, EPC*(k+1)) ----
    sel = (edges >= EPC * k) & (edges < EPC * (k + 1))
    e_l = edges[sel] - EPC * k
    v_g = vertex[sel]
    t = e_l >> 7                      # edge tile 0..24
    g = v_g & 3                       # stripe
    r = v_g >> 2                      # within-stripe gather row (< 25088)
    col = e_l & 127
    key = t * cfg.NS + g
    order = np.argsort(key, kind="stable")
    ks, rs_, cols = key[order], r[order], col[order]
    counts = np.bincount(ks, minlength=cfg.et * cfg.NS)
    assert counts.max() <= cfg.cap1, (counts.max(), cfg.cap1)
    starts = np.cumsum(counts) - counts
    rank = np.arange(len(ks)) - starts[ks]
    # gather indices [et, NS, cap1]; trailing pads -1 (ucode trims them)
    g1 = np.full((cfg.et, cfg.NS, cfg.cap1), -1 if PAD_NEG else 0, np.int64)
    g1[ks // cfg.NS, ks % cfg.NS, rank] = rs_
    # per-slot one-hot column offsets [et, 128, ch1p]; pads -1 (match nothing)
    pos = (ks % cfg.NS) * cfg.cap1 + rank
    off1 = np.full((cfg.et, 128, cfg.ch1p), -1.0, np.float32)
    off1[ks // cfg.NS, pos & 127, pos >> 7] = cols
    g1w = np.stack([
        np.concatenate([wrap_idx(g1[tt, gg]) for gg in range(cfg.NS)], axis=1)
        for tt in range(cfg.et)])

    homo_pad = np.zeros(EPC, np.float32)
    e0, e1 = EPC * k, min(EPC * (k + 1), cfg.E)
    homo_pad[0:e1 - e0] = np.asarray(homo)[e0:e1]
    homo_t = np.ascontiguousarray(homo_pad.reshape(cfg.et, 128).T)

    # ---- phase 2 (vertex shard): slots with vertex in [npc*k, npc*(k+1)) ----
    sel2 = (vertex >= npc * k) & (vertex < npc * (k + 1))
    v_l = vertex[sel2] - npc * k
    e_g = edges[sel2]                 # global edge id < 25000 (int16 ok)
    s = v_l >> 7
    col2 = v_l & 127
    order2 = np.argsort(s, kind="stable")
    ss, eg2, c2s = s[order2], e_g[order2], col2[order2]
    counts2 = np.bincount(ss, minlength=cfg.nt)
    assert counts2.max() <= cfg.cap2, (counts2.max(), cfg.cap2)
    starts2 = np.cumsum(counts2) - counts2
    rank2 = np.arange(len(ss)) - starts2[ss]
    g2 = np.full((cfg.nt, cfg.cap2), -1 if PAD_NEG else 0, np.int64)
    g2[ss, rank2] = eg2
    off2 = np.full((cfg.nt, 128, cfg.c2p), -1.0, np.float32)
    off2[ss, rank2 & 127, rank2 >> 7] = c2s
    g2w = np.stack([wrap_idx(g2[tt]) for tt in range(cfg.nt)])

    return {
        "xt": xt,
        "w65": w65,
        "homo_t": homo_t,
        "iota": iota,
        "g1": np.ascontiguousarray(g1w),
        "off1": np.ascontiguousarray(off1.astype(BF_NP)),
        "g2": np.ascontiguousarray(g2w),
        "off2": np.ascontiguousarray(off2.astype(BF_NP)),
    }


def build_nc(cfg: Cfg):
    c1, c2, ch1 = cfg.c1, cfg.c2, cfg.ch1
    nc = bacc.Bacc("TRN2", target_bir_lowering=False, debug=False,
                   num_devices=cfg.n_cores, num_swdge_queues=4)

    xt_d = nc.dram_tensor("xt", [65, cfg.npc], BF, kind="ExternalInput")
    w_d = nc.dram_tensor("w65", [65, 65], BF, kind="ExternalInput")
    homo_d = nc.dram_tensor("homo_t", [128, cfg.et], F32, kind="ExternalInput")
    iota_d = nc.dram_tensor("iota", [128, cfg.MW * 128], BF, kind="ExternalInput")
    g1_d = nc.dram_tensor("g1", [cfg.et, 128, cfg.NS * cfg.cap1 // 16], I16,
                          kind="ExternalInput")
    off1_d = nc.dram_tensor("off1", [cfg.et, 128, cfg.ch1p], BF,
                            kind="ExternalInput")
    g2_d = nc.dram_tensor("g2", [cfg.nt, 128, cfg.cap2 // 16], I16,
                          kind="ExternalInput")
    off2_d = nc.dram_tensor("off2", [cfg.nt, 128, cfg.c2p], BF,
                            kind="ExternalInput")
    out_d = nc.dram_tensor("out", [cfg.npc, 64], F32, kind="ExternalOutput")

    xptmy_d = nc.dram_tensor("XptMy", [cfg.npc, 128], BF, kind="Internal")
    xptfull_d = nc.dram_tensor("XptFull", [cfg.NV, 128], BF, kind="Internal",
                               addr_space="Shared")
    zetmy_d = nc.dram_tensor("ZetMy", [cfg.EPC, 128], BF, kind="Internal")
    zetfull_d = nc.dram_tensor("ZetFull", [cfg.EPAD, 128], BF, kind="Internal",
                               addr_space="Shared")

    iw = cfg.cap1 // 16  # idx cols per stripe (176)

    with tile.TileContext(nc) as tc:
        with (
            tc.tile_pool(name="const", bufs=1) as pc,
            tc.tile_pool(name="stg", bufs=4) as pstg,
            tc.tile_pool(name="idx", bufs=4) as pidx,
            tc.tile_pool(name="off", bufs=4) as poff,
            tc.tile_pool(name="mask", bufs=2) as pm,
            tc.tile_pool(name="gat1", bufs=2) as pg,
            tc.tile_pool(name="gat2", bufs=4) as pg2,
            tc.tile_pool(name="fin", bufs=4) as pf,
            tc.tile_pool(name="zo", bufs=3) as po,
            tc.tile_pool(name="psum", bufs=2, space="PSUM") as pp,
        ):
            xt_sb = pc.tile([65, cfg.npc], BF)
            nc.sync.dma_start(out=xt_sb[:], in_=xt_d[:])
            w_sb = pc.tile([65, 65], BF)
            nc.sync.dma_start(out=w_sb[:], in_=w_d[:])
            homo_sb = pc.tile([128, cfg.et], F32)
            nc.sync.dma_start(out=homo_sb[:], in_=homo_d[:])
            iota_sb = pc.tile([128, cfg.MW * 128], BF)
            nc.sync.dma_start(out=iota_sb[:], in_=iota_d[:])

            # memset gather pool buffers once: with -1-trimmed pads the pad
            # region is never written, so first-use SBUF garbage (possible
            # NaN bit patterns) must be cleared before it meets a matmul.
            for g in range(cfg.NS):
                for _ in range(2):
                    gt = pg.tile([128, c1, 128], BF, tag=f"g1_{g}")
                    nc.vector.memset(gt[:], 0.0)
            for _ in range(4):
                gt = pg2.tile([128, c2, 128], BF, tag="g2")
                nc.vector.memset(gt[:], 0.0)

            # ---- phase 0: Xp rows for my vertex shard ----
            for s in range(cfg.nt):
                ps = pp.tile([128, 65], F32, tag="ps0")
                nc.tensor.matmul(ps[:], lhsT=xt_sb[:, s * 128:(s + 1) * 128],
                                 rhs=w_sb[:], start=True, stop=True)
                stg = pstg.tile([128, 128], BF, tag="stg")
                nc.scalar.copy(out=stg[:, 0:65], in_=ps[:])
                nc.scalar.dma_start(out=xptmy_d[s * 128:(s + 1) * 128, :],
                                    in_=stg[:])

            nc.gpsimd.collective_compute(
                "AllGather", mybir.AluOpType.bypass,
                replica_groups=[list(range(cfg.n_cores))],
                ins=[xptmy_d.ap()], outs=[xptfull_d.ap()],
            )

            # striped views of XptFull: stripe g holds rows v with v%4==g
            stripes = [xptfull_d.ap().rearrange("(r g) c -> g r c", g=cfg.NS)[g]
                       for g in range(cfg.NS)]

            def build_masks(mk, off, nch):
                """mk[:, 0:nch*128] |= one-hot rows from per-slot offsets."""
                for b in range(0, nch, cfg.MW):
                    w = min(cfg.MW, nch - b)
                    nc.vector.tensor_tensor(
                        out=mk[:, b * 128:(b + w) * 128].rearrange(
                            "p (c j) -> p c j", j=128),
                        in0=iota_sb[:, 0:w * 128].rearrange(
                            "p (c j) -> p c j", j=128),
                        in1=off[:, b:b + w].unsqueeze(2).broadcast_to(
                            [128, w, 128]),
                        op=mybir.AluOpType.is_equal)

            # ---- phase 1: edge-tile accumulation ----
            for t in range(cfg.et):
                gi = pidx.tile([128, cfg.NS * iw], I16, tag="gi1")
                nc.sync.dma_start(out=gi[:], in_=g1_d[t])
                of = poff.tile([128, cfg.ch1p], BF, tag="of1")
                nc.sync.dma_start(out=of[:], in_=off1_d[t])
                mk = pm.tile([128, cfg.ch1p * 128], BF, tag="mk1")
                build_masks(mk, of, cfg.ch1p)
                gs = []
                for g in range(cfg.NS):
                    gt = pg.tile([128, c1, 128], BF, tag=f"g1_{g}")
                    nc.gpsimd.dma_gather(
                        gt[:], stripes[g], gi[:, g * iw:(g + 1) * iw],
                        cfg.cap1, cfg.cap1, 128, elem_step=cfg.NS * 128,
                        single_packet=False, queue_num=g)
                    gs.append(gt)
                ps = pp.tile([128, 65], F32, tag="ps1")
                for c in range(ch1):
                    nc.tensor.matmul(ps[:], lhsT=mk[:, c * 128:(c + 1) * 128],
                                     rhs=gs[c // c1][:, c % c1, 0:65],
                                     start=(c == 0), stop=(c == ch1 - 1))
                er = pf.tile([128, 65], F32, tag="er")
                nc.vector.tensor_copy(out=er[:], in_=ps[:])
                cntm = pf.tile([128, 1], F32, tag="cntm")
                nc.vector.tensor_scalar_max(out=cntm[:], in0=er[:, 64:65],
                                            scalar1=1.0)
                rec = pf.tile([128, 1], F32, tag="rec")
                nc.vector.reciprocal(out=rec[:], in_=cntm[:])
                scale = pf.tile([128, 1], F32, tag="scale")
                nc.vector.tensor_tensor(out=scale[:], in0=rec[:],
                                        in1=homo_sb[:, t:t + 1],
                                        op=mybir.AluOpType.mult)
                z = po.tile([128, 128], BF, tag="z")
                nc.vector.tensor_scalar_mul(out=z[:, 0:64], in0=er[:, 0:64],
                                            scalar1=scale[:])
                nc.vector.tensor_copy(out=z[:, 64:65], in_=homo_sb[:, t:t + 1])
                nc.scalar.dma_start(out=zetmy_d[t * 128:(t + 1) * 128, :],
                                    in_=z[:])

            nc.gpsimd.collective_compute(
                "AllGather", mybir.AluOpType.bypass,
                replica_groups=[list(range(cfg.n_cores))],
                ins=[zetmy_d.ap()], outs=[zetfull_d.ap()],
            )

            # ---- phase 2: node-tile accumulation + finalize ----
            for s in range(cfg.nt):
                gi = pidx.tile([128, cfg.cap2 // 16], I16, tag="gi2")
                nc.sync.dma_start(out=gi[:], in_=g2_d[s])
                of = poff.tile([128, cfg.c2p], BF, tag="of2")
                nc.sync.dma_start(out=of[:], in_=off2_d[s])
                mk = pm.tile([128, cfg.c2p * 128], BF, tag="mk2")
                build_masks(mk, of, cfg.c2p)
                gt = pg2.tile([128, c2, 128], BF, tag="g2")
                if P2_SPLIT is None:
                    nc.gpsimd.dma_gather(gt[:], zetfull_d.ap(), gi[:],
                                         cfg.cap2, cfg.cap2, 128,
                                         single_packet=False, queue_num=s % 4)
                else:
                    c0 = 0
                    for g, cg in enumerate(P2_SPLIT):
                        nc.gpsimd.dma_gather(
                            gt[:, c0:c0 + cg, :], zetfull_d.ap(),
                            gi[:, c0 * 8:(c0 + cg) * 8],
                            cg * 128, cg * 128, 128,
                            single_packet=False, queue_num=g)
                        c0 += cg
                ps = pp.tile([128, 65], F32, tag="ps2")
                for c in range(c2):
                    nc.tensor.matmul(ps[:], lhsT=mk[:, c * 128:(c + 1) * 128],
                                     rhs=gt[:, c, 0:65],
                                     start=(c == 0), stop=(c == c2 - 1))
                attm = pf.tile([128, 1], F32, tag="attm")
                nc.vector.tensor_scalar_max(out=attm[:], in0=ps[:, 64:65],
                                            scalar1=1e-30)
                arec = pf.tile([128, 1], F32, tag="arec")
                nc.vector.reciprocal(out=arec[:], in_=attm[:])
                o = pf.tile([128, 64], F32, tag="o")
                nc.scalar.mul(out=o[:], in_=ps[:, 0:64], mul=arec[:])
                xp_sb = pf.tile([128, 128], BF, tag="xp")
                nc.sync.dma_start(out=xp_sb[:],
                                  in_=xptmy_d[s * 128:(s + 1) * 128, :])
                xpf = pf.tile([128, 64], F32, tag="xpf")
                nc.scalar.copy(out=xpf[:], in_=xp_sb[:, 0:64])
                o2 = pf.tile([128, 64], F32, tag="o2")
                nc.vector.tensor_tensor(out=o2[:], in0=o[:], in1=xpf[:],
                                        op=mybir.AluOpType.add)
                sq = pf.tile([128, 64], F32, tag="sq")
                nc.vector.tensor_tensor(out=sq[:], in0=o2[:], in1=o2[:],
                                        op=mybir.AluOpType.mult)
                rs = pf.tile([128, 1], F32, tag="rs")
                nc.vector.reduce_sum(out=rs[:], in_=sq[:],
                                     axis=mybir.AxisListType.X)
                rn = pf.tile([128, 1], F32, tag="rn")
                nc.scalar.sqrt(out=rn[:], in_=rs[:])
                rnm = pf.tile([128, 1], F32, tag="rnm")
                nc.vector.tensor_scalar_max(out=rnm[:], in0=rn[:], scalar1=1e-30)
                rrec = pf.tile([128, 1], F32, tag="rrec")
                nc.vector.reciprocal(out=rrec[:], in_=rnm[:])
                ot = po.tile([128, 64], F32, tag="ot")
                nc.scalar.mul(out=ot[:], in_=o2[:], mul=rrec[:])
                nc.scalar.dma_start(out=out_d[s * 128:(s + 1) * 128, :],
                                    in_=ot[:])

    nc.compile()
    return nc


_NC_CACHE = {}


def kernel(**inputs) -> np.ndarray:
    """Full inputs in, full output out. Shards across 8 NeuronCores internally."""
    cfg = Cfg()
    X = np.asarray(inputs["X"], dtype=np.float32)
    W = np.asarray(inputs["W"], dtype=np.float32)
    homo = np.asarray(inputs["homo"], dtype=np.float32)
    vertex = np.asarray(inputs["vertex"])
    edges = np.asarray(inputs["edges"])
    assert X.shape == (cfg.N, 64) and homo.shape == (cfg.E,)

    if cfg not in _NC_CACHE:
        _NC_CACHE[cfg] = build_nc(cfg)
    nc = _NC_CACHE[cfg]

    in_maps = [prep_core_inputs(cfg, k, X, W, homo, vertex, edges)
               for k in range(cfg.n_cores)]
    res = bass_utils.run_bass_kernel_spmd(
        nc, in_maps, core_ids=list(range(cfg.n_cores)))
    out = np.concatenate(
        [res.results[k]["out"] for k in range(cfg.n_cores)], axis=0)
    return out[:cfg.N].astype(np.float32)
